# revision 4
# baseline (speedup 1.0000x reference)
"""Trainium2 Bass kernel for nn_LocalConnectivity (diamond-ring circular stencil).

out[i,j] = sum_{d=1..5} w_d * sum_{|di|+|dj|=d} x[(i+di)%H, (j+dj)%W]

Strategy: row-shard across 8 NeuronCores (512 rows each + 5-row circular
halo, columns pre-padded with 5-col circular halo on host). Per core the
61-tap stencil is computed on the TensorEngine as 11 banded matmuls (one
per column shift dj in [-5,5]): PSUM[m, c] += W_dj[k, m] * strip[k, c+5+dj]
where W_dj is a [128, 118] constant band matrix holding the vertical taps
for that dj and the column shift rides the rhs access pattern for free.
float32r matmuls stream at 1 cycle/row (vs 4 for float32) at ~2e-4 rel err.
"""
import numpy as np
from contextlib import ExitStack

import concourse.bass as bass
import concourse.tile as tile
from concourse import bacc, mybir
from concourse.bass_utils import run_bass_kernel_spmd

N_CORES = 8
H = W = 4096
MAXD = 5
ROWS_PER_CORE = H // N_CORES          # 512
IN_ROWS = ROWS_PER_CORE + 2 * MAXD    # 522
IN_COLS = W + 2 * MAXD                # 4106
NCOL = 512                            # matmul free dim (one PSUM bank, fp32 max)
NCHUNK = W // NCOL                    # 8
M_OUT = 118                           # output rows per row-window (K=128 - 2*MAXD)
# row windows: (input_row_start, out_row_start, K, M)
WINDOWS = []
_o = 0
while _o < ROWS_PER_CORE:
    m = min(M_OUT, ROWS_PER_CORE - _o)
    WINDOWS.append((_o, _o, m + 2 * MAXD, m))
    _o += m

_CACHE = {}


def _band_weights(distance_weights: np.ndarray) -> np.ndarray:
    """w_flat [128, 11*118]: w_flat[k, (dj+5)*118 + m] = K2d[k-m-5, dj]."""
    wd = np.asarray(distance_weights, dtype=np.float32)
    w = np.zeros((11, 128, M_OUT), dtype=np.float32)
    for dj in range(-MAXD, MAXD + 1):
        for di in range(-MAXD, MAXD + 1):
            d = abs(di) + abs(dj)
            if not (1 <= d <= MAXD):
                continue
            m = np.arange(M_OUT)
            k = m + MAXD + di
            ok = (k >= 0) & (k < 128)
            w[dj + MAXD, k[ok], m[ok]] = wd[d - 1]
    return np.ascontiguousarray(w.transpose(1, 0, 2).reshape(128, 11 * M_OUT))


def _build():
    dtr = mybir.dt.float32r
    dtf = mybir.dt.float32
    nc = bacc.Bacc("TRN2", target_bir_lowering=False, debug=False,
                   num_devices=N_CORES)
    x = nc.dram_tensor("x", [IN_ROWS, IN_COLS], dtr, kind="ExternalInput").ap()
    wts = nc.dram_tensor("w", [128, 11 * M_OUT], dtr, kind="ExternalInput").ap()
    y = nc.dram_tensor("y", [ROWS_PER_CORE, W], dtf, kind="ExternalOutput").ap()

    with tile.TileContext(nc) as tc, ExitStack() as ctx:
        spool = ctx.enter_context(tc.tile_pool(name="strip", bufs=2))
        wpool = ctx.enter_context(tc.tile_pool(name="wts", bufs=1))
        opool = ctx.enter_context(tc.tile_pool(name="out", bufs=4))
        ppool = ctx.enter_context(tc.tile_pool(name="ps", bufs=8, space="PSUM"))

        wt = wpool.tile([128, 11 * M_OUT], dtr)
        nc.scalar.dma_start(wt[:], wts[:])

        CMID = IN_COLS // 2
        for (in0, out0, kdim, m) in WINDOWS:
            # Split each strip load across two otherwise-idle DMA queues so
            # prefetch is never head-of-line blocked behind output stores.
            st = spool.tile([128, IN_COLS], dtr, tag="strip")
            nc.gpsimd.dma_start(st[:kdim, :CMID], x[in0:in0 + kdim, :CMID])
            nc.scalar.dma_start(st[:kdim, CMID:], x[in0:in0 + kdim, CMID:])
            for cc in range(NCHUNK):
                ps = ppool.tile([m, NCOL], dtf, tag="ps")
                for j, dj in enumerate(range(-MAXD, MAXD + 1)):
                    c0 = cc * NCOL + MAXD + dj
                    nc.tensor.matmul(
                        ps[:],
                        wt[:kdim, (dj + MAXD) * M_OUT:(dj + MAXD) * M_OUT + m],
                        st[:kdim, c0:c0 + NCOL],
                        start=(j == 0), stop=(j == 10),
                    )
                ot = opool.tile([m, NCOL], dtf, tag="out")
                if cc % 2 == 0:
                    nc.vector.tensor_copy(ot[:], ps[:])
                else:
                    nc.scalar.copy(ot[:], ps[:])
                nc.sync.dma_start(
                    y[out0:out0 + m, cc * NCOL:(cc + 1) * NCOL], ot[:])
    nc.compile()
    return nc


def kernel(grid_spikes: np.ndarray, distance_weights: np.ndarray) -> np.ndarray:
    x = np.ascontiguousarray(grid_spikes, dtype=np.float32)
    assert x.shape == (H, W)
    if "nc" not in _CACHE:
        _CACHE["nc"] = _build()
    nc = _CACHE["nc"]

    w_flat = _band_weights(distance_weights)
    xpad = np.concatenate([x[:, -MAXD:], x, x[:, :MAXD]], axis=1)
    in_maps = []
    for c in range(N_CORES):
        rows = np.arange(c * ROWS_PER_CORE - MAXD,
                         c * ROWS_PER_CORE + ROWS_PER_CORE + MAXD) % H
        in_maps.append({"x": np.ascontiguousarray(xpad[rows]), "w": w_flat})

    res = run_bass_kernel_spmd(nc, in_maps, list(range(N_CORES)))
    out = np.concatenate([res.results[c]["y"] for c in range(N_CORES)], axis=0)
    return out.astype(np.float32)
